# revision 36
# baseline (speedup 1.0000x reference)
"""Trainium2 Bass kernel: Mistral GQA attention block.

Full problem: B=2, S=2048, HIDDEN=4096, 32 Q heads / 8 KV heads, head_dim=128.
  out = (softmax(rope(XqWq) rope(XkWk)^T / sqrt(128)) (XvWv)) Wo

Sharding: 8 cores = (batch 2) x (head-group 4). Each core handles one batch
element and 8 Q heads / 2 KV heads: Wq/Wk/Wv split on output dim, Wo split on
input dim; host sums the 4 partial outputs per batch element.

Per-core layout strategy (all matmuls fp16 in / fp32 psum accumulate):
  * Host passes X^T (hidden on partitions) so Q^T/K^T [d, q] come straight
    out of the projection matmuls; scores are computed transposed
    S^T[k, q] = K^T^T(stationary) @ Q^T(moving) so softmax reduction over k
    is a ones-vector matmul and PV needs no transposes at all:
    ctx^T[d, q] = V(stationary [k,d]) @ E^T(moving), which is exactly the
    stationary operand the output projection needs.
  * exp via ScalarE activation with scale=1/sqrt(128) and a constant bias
    (cancels in normalization) to keep e^s comfortably inside fp16 range.
  * RoPE on [d, q] tiles: rotate_half is a signed 128x128 permutation
    matmul; cos/sin tables arrive from host transposed [d, q] in fp16.
"""

import math
from contextlib import ExitStack

import numpy as np

import concourse.bass as bass
import concourse.tile as tile
from concourse import bacc, bass_isa, mybir
from concourse.bass import ds
from concourse.bass_utils import run_bass_kernel_spmd

F16 = mybir.dt.float16
F32 = mybir.dt.float32
AF = mybir.ActivationFunctionType

HIDDEN = 4096
N_HEADS = 32
N_KV = 8
HD = 128
B = 2
S_FULL = 2048
NCORES = 8
GROUPS = 4                  # head groups = cores per batch element
HQ = N_HEADS // GROUPS      # 8 q heads per core
KVH = N_KV // GROUPS        # 2 kv heads per core
ROPE_THETA = 10000.0
SCALE = 1.0 / math.sqrt(HD)
EXP_BIAS = -6.0             # exp(s*SCALE + EXP_BIAS); cancels in softmax norm


def build_nc(S=S_FULL, E=HIDDEN, HQ_=HQ, KVH_=KVH, num_devices=NCORES,
             loop_iters=1, dbg=False):
    """Trace + compile the per-core program. All cores run the same program
    (SPMD); sharding is purely in which arrays each core receives."""
    assert S % 1024 == 0 and E % 128 == 0
    EC = E // 128            # contraction chunks
    KD = KVH_ * 128          # kv head block width (256)
    ST = S // 128            # 128-tiles along sequence
    SH = S // 2              # resident half of sequence
    SHC = SH // 512          # 512-chunks per half
    STH = ST // 2            # 128-tiles per half
    SC = S // 512            # 512-chunks of full sequence
    E8 = E // 512

    nc = bacc.Bacc("TRN2", target_bir_lowering=False, debug=False,
                   num_devices=num_devices)

    xq = nc.dram_tensor("xq", [E, S], F16, kind="ExternalInput")
    xk = nc.dram_tensor("xk", [E, S], F16, kind="ExternalInput")
    xv = nc.dram_tensor("xv", [E, S], F16, kind="ExternalInput")
    wq_d = nc.dram_tensor("wq", [HQ_, 128, EC * 128], F16, kind="ExternalInput")
    wk_d = nc.dram_tensor("wk", [KVH_, 128, EC * 128], F16, kind="ExternalInput")
    wv_d = nc.dram_tensor("wv", [128, EC, KD], F16, kind="ExternalInput")
    wo_d = nc.dram_tensor("wo", [128, E8, HQ_ * 512], F16, kind="ExternalInput")
    cos_d = nc.dram_tensor("cosT", [128, S], F16, kind="ExternalInput")
    sin_d = nc.dram_tensor("sinT", [128, S], F16, kind="ExternalInput")
    rmat_d = nc.dram_tensor("rmat", [128, 128], F16, kind="ExternalInput")
    out_d = nc.dram_tensor("out", [S, E], F32, kind="ExternalOutput")
    if dbg:
        dbg_qt = nc.dram_tensor("dbg_qt", [HQ_, 128, S], F16,
                                kind="ExternalOutput")
        dbg_kt = nc.dram_tensor("dbg_kt", [KVH_, 128, S], F16,
                                kind="ExternalOutput")
        dbg_v = nc.dram_tensor("dbg_v", [S // 128, 128, KVH_ * 128], F16,
                               kind="ExternalOutput")
        dbg_ctx = nc.dram_tensor("dbg_ctx", [HQ_, 128, S], F16,
                                 kind="ExternalOutput")
        dbg_ar = nc.dram_tensor("dbg_ar", [128, 512], F32,
                                kind="ExternalOutput")
        dbg_kraw = nc.dram_tensor("dbg_kraw", [KVH_ * S // 512, 128, 512],
                                  F16, kind="ExternalOutput")

    with tile.TileContext(nc) as tc:
        def body(_iv=None):
            with ExitStack() as stk:
                consts = stk.enter_context(
                    tc.tile_pool(name="consts", bufs=1))
                acts = stk.enter_context(
                    tc.tile_pool(name="acts", bufs=1))

                # Weights/constants go on the ScalarE HWDGE ring so they
                # don't queue behind the bulk X streams on the SP ring.
                rmat_sb = consts.tile([128, 128], F16, tag="rmat",
                                      name="rmat_sb")
                nc.scalar.dma_start(out=rmat_sb, in_=rmat_d[:, :])
                cos_sb = consts.tile([128, S], F16, tag="cos", name="cos_sb")
                sin_sb = consts.tile([128, S], F16, tag="sin", name="sin_sb")
                bias_sb = consts.tile([128, 1], F32, tag="ebias",
                                      name="bias_sb")
                nc.vector.memset(bias_sb, EXP_BIAS)

                # Per-core activations that persist across phases.
                qt_sb = [acts.tile([128, S], F16, tag=f"qt{h}", name=f"qt{h}")
                         for h in range(HQ_)]
                kt_sb = [acts.tile([128, S], F16, tag=f"kt{d}", name=f"kt{d}")
                         for d in range(KVH_)]
                v_sb = [acts.tile([128, KD], F16, tag=f"v{t}", name=f"v{t}")
                        for t in range(ST)]

                # ---------------- QKV projections + RoPE -----------------
                with tc.tile_pool(name="xres", bufs=1) as xpool, \
                     tc.tile_pool(name="wstr", bufs=2) as wpool, \
                     tc.tile_pool(name="ropet", bufs=3) as rtmp, \
                     tc.tile_pool(name="wkv", bufs=1) as wkvp:

                    def w_stream(w_dram, d):
                        w_sb = wpool.tile([128, EC * 128], F16, tag="wq",
                                          name="w_sb")
                        nc.scalar.dma_start(out=w_sb, in_=w_dram[d])
                        return lambda ec: w_sb[:, ds(ec * 128, 128)]

                    wv_sb = wkvp.tile([128, EC, KD], F16, tag="wv",
                                      name="wv_sb")

                    def load_half(x_dram, half):
                        xh = []
                        for ec in range(EC):
                            t = xpool.tile([128, SH], F16, tag=f"x{ec}",
                                           name=f"xh{ec}")
                            nc.sync.dma_start(
                                out=t,
                                in_=x_dram[ds(ec * 128, 128), ds(half * SH, SH)])
                            xh.append(t)
                        return xh

                    # Deferred RoPE application so PE never waits on the
                    # ScalarE psum->sbuf copy: rope of chunk i is emitted
                    # after the projection matmuls of chunk i+1.
                    pending_rope = []

                    def rope_chunk(dst, src_ps, off, pool, dbg_slot=None):
                        qraw = rtmp.tile([128, 512], F16, tag="qraw",
                                         name="qraw")
                        nc.scalar.copy(qraw, src_ps)
                        if dbg and dbg_slot is not None:
                            nc.sync.dma_start(out=dbg_kraw[dbg_slot],
                                              in_=qraw)
                        rot = pool.tile([128, 512], F32, tag="rot", bufs=2,
                                        name="rot")
                        nc.tensor.matmul(rot, rmat_sb, qraw,
                                         start=True, stop=True)
                        t1 = rtmp.tile([128, 512], F16, tag="t1", name="t1")
                        nc.vector.tensor_mul(t1, qraw, cos_sb[:, ds(off, 512)])
                        t2 = rtmp.tile([128, 512], F16, tag="t2", name="t2")
                        nc.vector.tensor_mul(t2, rot, sin_sb[:, ds(off, 512)])
                        nc.vector.tensor_add(dst, t1, t2)

                    def flush_rope():
                        while pending_rope:
                            rope_chunk(*pending_rope.pop(0))

                    # K projection: ec-outer (all 2*SHC psum chains advance
                    # as each X tile lands) so the cold-start DMA stream
                    # paces PE instead of stalling it chain-by-chain.
                    # NOTE: every dma_start must be EMITTED before any
                    # instruction that reads its destination — Tile resolves
                    # dependencies by program order, so a later-emitted DMA
                    # would be scheduled AFTER those reads (reads would see
                    # uninitialized SBUF).
                    with tc.tile_pool(name="ppk", bufs=1,
                                      space="PSUM") as ppk:
                        w_sbs = [w_stream(wk_d, d) for d in range(KVH_)]
                        nc.scalar.dma_start(out=cos_sb, in_=cos_d[:, :])
                        nc.scalar.dma_start(out=sin_sb, in_=sin_d[:, :])
                        nc.scalar.dma_start(out=wv_sb, in_=wv_d[:, :, :])
                        for half in range(2):
                            xh = load_half(xk, half)
                            chains = [
                                ppk.tile([128, 512], F32, tag="projk",
                                         bufs=KVH_ * SHC + 1, name="pk")
                                for _ in range(KVH_ * SHC)]
                            for ec in range(EC):
                                for d in range(KVH_):
                                    for cc in range(SHC):
                                        nc.tensor.matmul(
                                            chains[d * SHC + cc],
                                            w_sbs[d](ec),
                                            xh[ec][:, ds(cc * 512, 512)],
                                            start=(ec == 0),
                                            stop=(ec == EC - 1))
                            for d in range(KVH_):
                                for cc in range(SHC):
                                    off = half * SH + cc * 512
                                    flush_rope()
                                    slot = (d * 2 * SHC + half * SHC + cc
                                            if dbg else None)
                                    pending_rope.append(
                                        (kt_sb[d][:, ds(off, 512)],
                                         chains[d * SHC + cc], off, ppk,
                                         slot))
                        flush_rope()

                    with tc.tile_pool(name="ppv", bufs=1,
                                      space="PSUM") as ppv:
                        for half in range(2):
                            xh = load_half(xv, half)
                            for ktg in range(STH // 4):
                                pss = []
                                for j in range(4):
                                    vp = ppv.tile([128, KD], F32, tag="vps",
                                                  bufs=6, name="vps")
                                    pss.append(vp)
                                for ec in range(EC):
                                    for j in range(4):
                                        kt_local = ktg * 4 + j
                                        nc.tensor.matmul(
                                            pss[j],
                                            xh[ec][:, ds(kt_local * 128, 128)],
                                            wv_sb[:, ec, :],
                                            start=(ec == 0),
                                            stop=(ec == EC - 1))
                                for j in range(4):
                                    kt_g = half * STH + ktg * 4 + j
                                    nc.scalar.copy(v_sb[kt_g], pss[j])

                    with tc.tile_pool(name="ppq", bufs=1,
                                      space="PSUM") as ppq:
                        for half in range(2):
                            xh = load_half(xq, half)
                            for d in range(HQ_):
                                w_sb = w_stream(wq_d, d)
                                for cc in range(SHC):
                                    ps = ppq.tile([128, 512], F32,
                                                  tag="proj", bufs=2,
                                                  name="proj_ps")
                                    for ec in range(EC):
                                        nc.tensor.matmul(
                                            ps,
                                            w_sb(ec),
                                            xh[ec][:, ds(cc * 512, 512)],
                                            start=(ec == 0),
                                            stop=(ec == EC - 1))
                                    off = half * SH + cc * 512
                                    flush_rope()
                                    pending_rope.append(
                                        (qt_sb[d][:, ds(off, 512)], ps, off,
                                         ppq))
                        flush_rope()

                # ------------- attention + output projection -------------
                # c-major: once all heads finish a 512-column chunk of ctx,
                # the 4 corresponding output-projection qtiles are emitted,
                # giving PE dense matmul work while ScalarE (exp-bound)
                # works on the next attention chunk.
                ctxp = stk.enter_context(
                    tc.tile_pool(name="ctxp", bufs=1, side="right"))
                wop = stk.enter_context(
                    tc.tile_pool(name="wop", bufs=1, side="right"))
                ctx_sb = [ctxp.tile([128, S], F16, tag=f"ctx{h}",
                                    name=f"ctx{h}")
                          for h in range(HQ_)]
                # wo arrives e8-major in 1 MB slices so the first out-proj
                # qtile only waits for slice 0.
                wo_sb = wop.tile([128, E8, HQ_ * 512], F16, tag="wo",
                                 name="wo_sb")
                for e8 in range(E8):
                    nc.scalar.dma_start(out=wo_sb[:, e8, :], in_=wo_d[:, e8, :])

                with tc.tile_pool(name="asb", bufs=3) as asb, \
                     tc.tile_pool(name="osb", bufs=4) as osb, \
                     tc.tile_pool(name="ap", bufs=2, space="PSUM") as ap:

                    def tree_sum(tiles):
                        """Pairwise DVE adds: fp16 levels, fp32 final.
                        Returns the [128, 512] fp32 column-partial sum."""
                        lvl = 1
                        while len(tiles) > 2:
                            nxt = []
                            for j in range(0, len(tiles), 2):
                                t = asb.tile([128, 512], F16,
                                             tag=f"tp{lvl}",
                                             bufs=3, name=f"tp{lvl}")
                                nc.vector.tensor_add(t, tiles[j],
                                                     tiles[j + 1])
                                nxt.append(t)
                            tiles = nxt
                            lvl += 1
                        acc = asb.tile([128, 512], F32, tag="acc",
                                       bufs=2, name="acc")
                        nc.vector.tensor_add(acc, tiles[0], tiles[1])
                        return acc

                    def out_mms(qtiles):
                        """Generator: one output-projection matmul per
                        next() — interleaved into attention chunks as PE
                        filler while ScalarE works on exp."""
                        for qt_i in qtiles:
                            for e8 in range(E8):
                                o_ps = ap.tile([128, 512], F32, tag="o",
                                               bufs=2, name="o_ps")
                                for h in range(HQ_):
                                    nc.tensor.matmul(
                                        o_ps,
                                        ctx_sb[h][:, ds(qt_i * 128, 128)],
                                        wo_sb[:, e8, ds(h * 512, 512)],
                                        start=(h == 0),
                                        stop=(h == HQ_ - 1))
                                    yield
                                ob = osb.tile([128, 512], F32, tag="ob",
                                              name="ob")
                                nc.vector.tensor_copy(ob, o_ps)
                                nc.sync.dma_start(
                                    out=out_d[ds(qt_i * 128, 128),
                                              ds(e8 * 512, 512)],
                                    in_=ob)

                    def attn_chunk(h, c, filler):
                        kv = h // (HQ_ // KVH_)
                        ctx_ps = ap.tile([128, 512], F32, tag="ctx",
                                         bufs=2, name="ctx_ps")
                        # software pipeline: s/exp of kt+1 before pv of
                        # kt, plus out-proj filler matmuls, so PE never
                        # waits on ScalarE.
                        es = []
                        for kt in range(ST + 1):
                            if kt < ST:
                                s_ps = ap.tile([128, 512], F32, tag="s",
                                               bufs=4, name="s_ps")
                                nc.tensor.matmul(
                                    s_ps,
                                    kt_sb[kv][:, ds(kt * 128, 128)],
                                    qt_sb[h][:, ds(c * 512, 512)],
                                    start=True, stop=True)
                                e_sb = asb.tile([128, 512], F16, tag="e",
                                                bufs=5, name="e_sb")
                                nc.scalar.activation(
                                    e_sb, s_ps, AF.Exp,
                                    bias=bias_sb, scale=SCALE)
                                es.append(e_sb)
                            if kt > 0:
                                nc.tensor.matmul(
                                    ctx_ps,
                                    v_sb[kt - 1][:, ds(kv * 128, 128)],
                                    es[kt - 1],
                                    start=(kt == 1), stop=(kt == ST))
                            for _ in range(2):
                                next(filler, None)
                        # softmax denominator: DVE tree + GPSIMD partition
                        # reduce; PE is not involved.
                        acc = tree_sum(es)
                        allred = asb.tile([128, 512], F32, tag="allred",
                                          bufs=2, name="allred")
                        nc.gpsimd.partition_all_reduce(
                            allred, acc, channels=128,
                            reduce_op=bass_isa.ReduceOp.add)
                        recip = asb.tile([128, 512], F32, tag="recip",
                                         bufs=2, name="recip")
                        nc.vector.reciprocal(recip, allred)
                        nc.vector.tensor_mul(
                            ctx_sb[h][:, ds(c * 512, 512)], ctx_ps, recip)
                        if dbg and h == 0 and c == 0:
                            nc.sync.dma_start(out=dbg_ar[:, :], in_=allred)

                    filler = iter(())
                    for c in range(SC):
                        for h in range(HQ_):
                            attn_chunk(h, c, filler)
                        # out-proj of chunk c interleaves into chunk c+1's
                        # attention (ctx of chunk c is then long finished).
                        for _ in filler:
                            pass
                        filler = out_mms(range(c * 4, c * 4 + 4))
                    for _ in filler:
                        pass

                    if dbg:
                        for h in range(HQ_):
                            nc.sync.dma_start(out=dbg_qt[h], in_=qt_sb[h])
                            nc.sync.dma_start(out=dbg_ctx[h], in_=ctx_sb[h])
                        for d in range(KVH_):
                            nc.sync.dma_start(out=dbg_kt[d], in_=kt_sb[d])
                        for t in range(S // 128):
                            nc.sync.dma_start(out=dbg_v[t], in_=v_sb[t])

        if loop_iters > 1:
            with tc.For_i(0, loop_iters, 1) as _i:
                body(_i)
        else:
            body()

    nc.compile()
    return nc


# ---------------------------------------------------------------------------
# Host side
# ---------------------------------------------------------------------------

def _rope_tables(position_ids_b, S):
    """cos/sin transposed [head_dim, S] in fp16 (rows 64..127 repeat 0..63)."""
    pos = np.asarray(position_ids_b, dtype=np.float64)
    inv_freq = 1.0 / (ROPE_THETA ** (np.arange(0, HD, 2, dtype=np.float64) / HD))
    freqs = inv_freq[:, None] * pos[None, :]          # [64, S]
    cosT = np.concatenate([np.cos(freqs)] * 2, axis=0).astype(np.float16)
    sinT = np.concatenate([np.sin(freqs)] * 2, axis=0).astype(np.float16)
    return np.ascontiguousarray(cosT), np.ascontiguousarray(sinT)


def _rmat():
    """lhsT of the rotate_half permutation: out = rmat.T @ x."""
    r = np.zeros((128, 128), np.float16)
    r[np.arange(64, 128), np.arange(0, 64)] = -1.0
    r[np.arange(0, 64), np.arange(64, 128)] = 1.0
    return r


def make_in_maps(query, key, value, position_ids, Wq, Wk, Wv, Wo,
                 S=S_FULL, E=HIDDEN, HQ_=HQ, KVH_=KVH, groups=GROUPS):
    EC = E // 128
    KD = KVH_ * 128
    rmat = _rmat()
    in_maps = []
    for b in range(query.shape[0]):
        xqT = np.ascontiguousarray(
            np.asarray(query[b], np.float32).T.astype(np.float16))
        xkT = np.ascontiguousarray(
            np.asarray(key[b], np.float32).T.astype(np.float16))
        xvT = np.ascontiguousarray(
            np.asarray(value[b], np.float32).T.astype(np.float16))
        cosT, sinT = _rope_tables(position_ids[b], S)
        for g in range(groups):
            wq_g = np.asarray(Wq[:, g * HQ_ * HD:(g + 1) * HQ_ * HD], np.float32)
            wq_l = np.ascontiguousarray(
                wq_g.reshape(EC, 128, HQ_, HD).transpose(2, 1, 0, 3)
                .reshape(HQ_, 128, EC * HD).astype(np.float16))
            wk_g = np.asarray(Wk[:, g * KD:(g + 1) * KD], np.float32)
            wk_l = np.ascontiguousarray(
                wk_g.reshape(EC, 128, KVH_, HD).transpose(2, 1, 0, 3)
                .reshape(KVH_, 128, EC * HD).astype(np.float16))
            wv_g = np.asarray(Wv[:, g * KD:(g + 1) * KD], np.float32)
            wv_l = np.ascontiguousarray(
                wv_g.reshape(EC, 128, KD).transpose(1, 0, 2).astype(np.float16))
            wo_g = np.asarray(Wo[g * HQ_ * HD:(g + 1) * HQ_ * HD, :], np.float32)
            wo_l = np.ascontiguousarray(
                wo_g.reshape(HQ_, 128, E // 512, 512).transpose(1, 2, 0, 3)
                .reshape(128, E // 512, HQ_ * 512).astype(np.float16))
            in_maps.append(dict(
                xq=xqT, xk=xkT, xv=xvT,
                wq=wq_l, wk=wk_l, wv=wv_l, wo=wo_l,
                cosT=cosT, sinT=sinT, rmat=rmat))
    return in_maps


_compiled_nc = None


def kernel(query, key, value, position_ids, Wq, Wk, Wv, Wo):
    global _compiled_nc
    if _compiled_nc is None:
        _compiled_nc = build_nc()
    nc = _compiled_nc

    in_maps = make_in_maps(query, key, value, position_ids, Wq, Wk, Wv, Wo)
    res = run_bass_kernel_spmd(nc, in_maps, core_ids=list(range(NCORES)))

    nb, S = query.shape[0], query.shape[1]
    out = np.zeros((nb, S, HIDDEN), np.float32)
    for b in range(nb):
        for g in range(GROUPS):
            out[b] += res.results[b * GROUPS + g]["out"]
    return out
